# revision 33
# baseline (speedup 1.0000x reference)
"""AdaptiveVectorModifier Trainium2 kernel (8 NeuronCores, data-parallel over rows).

Reference computation (per row n of x flattened to (N=8192, V=2048)):
    feats = x @ W_map.T                  (N, 128)
    h     = silu(feats @ W1.T + b1)      (N, 512)
    A     = (h @ W2.T + b2)              (N, 128, 128)
    feats2= einsum('nij,nj->ni', A, feats)
    out   = x + feats2 @ W_map

Sharding: rows split 8 ways (1024 rows/core); weights replicated.

Everything on-chip is computed in "transposed space" (rows on the free dim)
so that every matmul contraction dim lands on SBUF partitions:
    s1: featsT (m,n)  = sum_v W_mapT[v,m] xT[v,n]           bf16
    s2: hT     (k,n)  = silu(sum_m W1T[m,k] featsT[m,n]+b1) bf16
    s3: A_t    (j,n)  = sum_k W2T[k, 128t+j] hT[k,n]        bf16 (t = i index)
    s4: P_t    (j,n)  = (A_t + b2[128t+j]) * featsT[j,n]
        feats2_nat[n,t] = sum_j P_t[j,n]                    N=1 matmuls vs ones
    s5: modT   (v,n)  = sum_i W_map[i,v] feats2T[i,n];  outT = modT + xT
Host pre-tiles every DRAM tensor so each DMA reads/writes contiguous
per-partition blocks (strided DMAs + per-DMA completion serialization on the
HWDGE rings were costing ~4us per transfer), and un-tiles the output.

Scheduling notes (the TensorE stream must stay dense — HAM re-throttles the
PE clock to 1.2 GHz after ~3.4us of idleness):
  - bulk W2T (16 MiB) streams on the gpsimd SWDGE ring (first 2 groups on the
    scalar HWDGE ring since the t-loop needs them ~15us in).
  - s1/s2 of block 1 and transpose/s5 of block 0 are interleaved into the
    t-loops so the PE never waits at phase boundaries.
  - stage-4 evac+multiply alternates between DVE (fused scalar_tensor_tensor)
    and ScalarE-evac + DVE-mul by t parity to balance the two engines.
"""

import numpy as np
import ml_dtypes

import concourse.bass as bass
import concourse.mybir as mybir
import concourse.tile as tile
from concourse import bacc
from concourse.masks import make_identity
from concourse.tile import add_dep_helper

F32 = mybir.dt.float32
BF16 = mybir.dt.bfloat16
AF = mybir.ActivationFunctionType
ALU = mybir.AluOpType

V = 2048     # vector dim
M = 128      # mod dim
K = 512      # hidden (4*M)
NL = 1024    # rows per core
NB = 512     # rows per block
N_CORES = 8
SKEW = 2     # s3 -> reduce software-pipeline skew (t-loop)
QG = 16      # W2T column groups (1 MiB each), group g covers t in [8g, 8g+8)


def build_graph(n_rows=NL, silu_via_sigmoid=False):
    assert n_rows % NB == 0
    nblk = n_rows // NB

    nc = bacc.Bacc(None, target_bir_lowering=False)

    VC = V // M            # 16 chunks of the vector dim
    KC = K // M            # 4 chunks of the hidden dim
    NCH = NB // M          # 4 row-chunks per block (for the s4 reduce)
    qg = (M * M) // QG

    # host-pre-tiled layouts: every DMA is contiguous per partition.
    # wpk packs all small weights into one DMA (per-DMA completion costs
    # ~3-5us; 4 small DMAs were serializing the scalar engine's stream):
    # per partition: [w_map 2048 bf16 | w1T 512 bf16 | b2r 128 f32 | b1c 4 f32]
    WPK = V + K + 2 * M + 2 * (K // M)  # in bf16 elements
    xtb_d = nc.declare_dram_parameter("xtb", [M, nblk, VC, NB], BF16, isOutput=False)
    w_mapT_d = nc.declare_dram_parameter("w_mapT", [M, VC, M], BF16, isOutput=False)
    wpk_d = nc.declare_dram_parameter("wpk", [M, WPK], BF16, isOutput=False)
    w2t_d = nc.declare_dram_parameter("w2t", [QG, M, KC, qg], BF16, isOutput=False)
    out_d = nc.declare_dram_parameter("out", [nblk, VC, M, NB], F32, isOutput=True)

    with tile.TileContext(nc) as tc:
        with (
            tc.tile_pool(name="weights", bufs=1) as wpool,
            tc.tile_pool(name="xtb", bufs=2) as xtb_pool,
            tc.tile_pool(name="featsT", bufs=2) as f_pool,
            tc.tile_pool(name="hT", bufs=2) as h_pool,
            tc.tile_pool(name="asb", bufs=2) as a_pool,
            tc.tile_pool(name="p", bufs=SKEW + 2) as p_pool,
            tc.tile_pool(name="f2", bufs=2) as f2_pool,
            tc.tile_pool(name="ot", bufs=3) as o_pool,
            tc.tile_pool(name="apsum", bufs=SKEW + 1, space=bass.MemorySpace.PSUM) as a_ps,
            tc.tile_pool(name="f2psum", bufs=2, space=bass.MemorySpace.PSUM) as f2_ps,
            tc.tile_pool(name="smallps", bufs=3, space=bass.MemorySpace.PSUM) as s_ps,
        ):
            # ---- scalar HWDGE ring: ONLY [w_mapT, w2T g0, g1] so the engine
            #      is free for ACTIVATEs right after ----
            w_mapT_sb = wpool.tile([M, VC, M], BF16, tag="w_mapT")
            nc.scalar.dma_start(w_mapT_sb[:], w_mapT_d[:])

            w2T_sb = wpool.tile([M, KC, M * M], BF16, tag="w2T")
            for g in range(2):
                nc.scalar.dma_start(
                    w2T_sb[:, :, g * qg : (g + 1) * qg], w2t_d[g]
                )

            # ---- packed small weights: one DMA on the sync ring ----
            wpk_sb = wpool.tile([M, WPK], BF16, tag="wpk")
            nc.sync.dma_start(wpk_sb[:], wpk_d[:])
            w_map_sb = wpk_sb[:, :V]
            w1T_sb = wpk_sb[:, V : V + K]
            b2r_sb = wpk_sb[:, V + K : V + K + 2 * M].bitcast(F32)
            b1_sb = wpk_sb[:, V + K + 2 * M :].bitcast(F32)

            ones_sb = wpool.tile([M, 1], BF16, tag="ones")
            nc.vector.memset(ones_sb[:], 1.0)
            # preload the SILU ACT table so the 1.3us table swap isn't in the
            # s2 critical path
            silu_warm = wpool.tile([M, 1], BF16, tag="silu_warm")
            nc.scalar.activation(silu_warm[:], ones_sb[:], AF.Silu)

            # ---- gpsimd SWDGE ring: W2T groups 2..15. The first is gated on
            #      s1 finishing (set below) so the 14 MiB bulk stream doesn't
            #      steal SDMA bandwidth from the startup-critical loads;
            #      the rest follow in SWDGE FIFO order. ----
            w2T_bulk_dmas = []
            for g in range(2, QG):
                w2T_bulk_dmas.append(
                    nc.gpsimd.dma_start(
                        w2T_sb[:, :, g * qg : (g + 1) * qg], w2t_d[g]
                    )
                )

            # identity is first needed at transpose time (>150us in); emit its
            # gpsimd ops after the W2T stream so they don't delay the groups
            ident_sb = wpool.tile([M, M], F32, tag="ident")
            make_identity(nc, ident_sb[:])

            # ---- x block loads: two contiguous 1 MiB DMAs per block (sync) ----
            xtb_tiles = {}

            def emit_xtb_load(nb):
                xtb = xtb_pool.tile([M, VC, NB], BF16, tag="xtb")
                half = VC // 2
                nc.sync.dma_start(xtb[:, :half, :], xtb_d[:, nb, :half, :])
                nc.sync.dma_start(xtb[:, half:, :], xtb_d[:, nb, half:, :])
                xtb_tiles[nb] = xtb

            emit_xtb_load(0)

            featsT = {}
            hT = {}
            feats_psums = {}

            def emit_s1_group(nb, q, nq=4):
                if q == 0:
                    feats_psums[nb] = s_ps.tile(
                        [M, NB], F32, tag="smallps", name="feats_psum"
                    )
                last = None
                for c in range(nq):
                    vc = q * nq + c
                    last = nc.tensor.matmul(
                        feats_psums[nb][:],
                        w_mapT_sb[:, vc, :],
                        xtb_tiles[nb][:, vc, :],
                        start=(vc == 0),
                        stop=(vc == VC - 1),
                    )
                return last

            def emit_s2(nb):
                fT = f_pool.tile([M, NB], BF16, tag="featsT")
                nc.scalar.activation(fT[:], feats_psums[nb][:], AF.Copy)
                featsT[nb] = fT

                hh = h_pool.tile([M, KC, NB], BF16, tag="hT")
                for kc in range(KC):
                    h_psum = s_ps.tile([M, NB], F32, tag="smallps")
                    nc.tensor.matmul(
                        h_psum[:],
                        w1T_sb[:, kc * M : (kc + 1) * M],
                        fT[:],
                        start=True,
                        stop=True,
                    )
                    if silu_via_sigmoid:
                        # CoreSim has no Silu LUT; emulate z*sigmoid(z)
                        sg = h_pool.tile([M, NB], BF16, tag="sg")
                        nc.scalar.activation(
                            sg[:], h_psum[:], AF.Sigmoid, bias=b1_sb[:, kc : kc + 1]
                        )
                        nc.vector.tensor_mul(hh[:, kc, :], sg[:], h_psum[:])
                    else:
                        nc.scalar.activation(
                            hh[:, kc, :], h_psum[:], AF.Silu, bias=b1_sb[:, kc : kc + 1]
                        )
                hT[nb] = hh

            def emit_s1_s2(nb):
                last = None
                for q in range(VC // 4):
                    last = emit_s1_group(nb, q)
                emit_s2(nb)
                return last

            f2n_psums = {}
            feats2T = {}

            def emit_transpose(nb, c, ps_pool, ps_tag):
                # feats2_nat chunk c -> feats2T columns [c*M, (c+1)*M)
                if c == 0:
                    f2n = f2_pool.tile([M, NCH, M], F32, tag="f2nat")
                    f2T = f2_pool.tile([M, NB], BF16, tag="feats2T")
                    emit_transpose.cur = (f2n, f2T)
                f2n, f2T = emit_transpose.cur
                nc.scalar.activation(f2n[:, c, :], f2n_psums[nb][:, c, :], AF.Copy)
                tr_psum = ps_pool.tile([M, M], F32, tag=ps_tag, name="tr_psum")
                nc.tensor.transpose(tr_psum[:], f2n[:, c, :], ident_sb[:])
                nc.scalar.activation(f2T[:, c * M : (c + 1) * M], tr_psum[:], AF.Copy)
                if c == NCH - 1:
                    feats2T[nb] = f2T

            def emit_s5(nb, vc, alt_pool=False):
                if alt_pool and vc % 2 == 1:
                    mod_psum = f2_ps.tile([M, NB], F32, tag="f2psum", name="mod_psum")
                else:
                    mod_psum = s_ps.tile([M, NB], F32, tag="smallps", name="mod_psum")
                nc.tensor.matmul(
                    mod_psum[:],
                    w_map_sb[:, vc * M : (vc + 1) * M],
                    feats2T[nb][:],
                    start=True,
                    stop=True,
                )
                ot = o_pool.tile([M, NB], F32, tag="ot")
                # residual add from the bf16 x tiles (still ~30x under the
                # accuracy gate; saves re-reading x in f32)
                nc.vector.tensor_add(ot[:], mod_psum[:], xtb_tiles[nb][:, vc, :])
                eng = nc.sync if vc % 2 == 0 else nc.scalar
                eng.dma_start(out_d[nb, vc], ot[:])

            def emit_tloop(nb, extra):
                """s3 + s4 software-pipelined t-loop; `extra` maps t -> list of
                emit-closures injected between iterations (deferred work from
                other phases, placed where its inputs are long since ready)."""
                fT = featsT[nb]
                hh = hT[nb]
                f2n_psum = f2_ps.tile([M, NCH, M], F32, tag="f2psum")
                f2n_psums[nb] = f2n_psum
                p_tiles = {}
                for tt in range(M + SKEW):
                    if tt < M:
                        a_psum = a_ps.tile([M, NB], F32, tag="apsum")
                        for kc in range(KC):
                            nc.tensor.matmul(
                                a_psum[:],
                                w2T_sb[:, kc, tt * M : (tt + 1) * M],
                                hh[:, kc, :],
                                start=(kc == 0),
                                stop=(kc == KC - 1),
                            )
                        p_sb = p_pool.tile([M, NB], BF16, tag="p")
                        if tt % 2 == 0:
                            # fused (A + b2) * featsT on DVE (PSUM operand, 1x)
                            nc.vector.scalar_tensor_tensor(
                                p_sb[:],
                                a_psum[:],
                                b2r_sb[:, tt : tt + 1],
                                fT[:],
                                op0=ALU.add,
                                op1=ALU.mult,
                            )
                        else:
                            # ScalarE evac (+b2, ->bf16), then DVE mul at 2x
                            a_sb = a_pool.tile([M, NB], BF16, tag="asb")
                            nc.scalar.activation(
                                a_sb[:], a_psum[:], AF.Identity,
                                bias=b2r_sb[:, tt : tt + 1],
                            )
                            nc.vector.tensor_mul(p_sb[:], a_sb[:], fT[:])
                        p_tiles[tt] = p_sb
                    if tt >= SKEW:
                        t0 = tt - SKEW
                        p_prev = p_tiles.pop(t0)
                        for c in range(NCH):
                            nc.tensor.matmul(
                                f2n_psum[:, c, t0 : t0 + 1],
                                p_prev[:, c * M : (c + 1) * M],
                                ones_sb[:],
                                start=True,
                                stop=True,
                            )
                    for fn in extra.get(tt, ()):
                        fn()

            # ---- emit: s1/s2(0); t-loop(0) with s1/s2(1) injected at t=16..;
            #      t-loop(1) with trans(0) at t=0..3 and s5(0) spread t=8..40;
            #      then trans(1) + s5(1) ----
            s1_last = emit_s1_s2(0)
            # release the W2T bulk stream only once startup-critical loads are
            # done (s1 finishing implies xtb+w_mapT have landed). Gate EVERY
            # group — the Tile scheduler reorders the gpsimd queue, so gating
            # just the first one lets the rest jump ahead.
            for dma in w2T_bulk_dmas:
                add_dep_helper(
                    dma.ins,
                    s1_last.ins,
                    sync=True,
                    reason="delay W2T bulk stream past startup-critical DMAs",
                )
            if nblk == 1:
                emit_tloop(0, {})
                for c in range(NCH):
                    emit_transpose(0, c, a_ps, "apsum")
                for vc in range(VC):
                    emit_s5(0, vc)
            else:
                assert nblk == 2
                extra0 = {0: [lambda: emit_xtb_load(1)]}
                for q in range(VC // 4):
                    extra0.setdefault(16 + 2 * q, []).append(
                        lambda q=q: emit_s1_group(1, q)
                    )
                extra0.setdefault(26, []).append(lambda: emit_s2(1))
                emit_tloop(0, extra0)
                extra = {
                    c: [lambda c=c: emit_transpose(0, c, s_ps, "smallps")]
                    for c in range(NCH)
                }
                for vc in range(VC):
                    extra.setdefault(8 + 2 * vc, []).append(
                        lambda vc=vc: emit_s5(0, vc)
                    )
                emit_tloop(1, extra)
                for c in range(NCH):
                    emit_transpose(1, c, a_ps, "apsum")
                for vc in range(VC):
                    emit_s5(1, vc, alt_pool=True)

    nc.compile()
    return nc


def make_in_maps(x, W_map, W1, b1, W2, b2, n_cores=N_CORES):
    xf = np.ascontiguousarray(x, dtype=np.float32).reshape(-1, V)
    n_rows = xf.shape[0] // n_cores
    nblk = n_rows // NB
    VC = V // M
    KC = K // M
    qg = (M * M) // QG
    bf = ml_dtypes.bfloat16

    W2T = np.ascontiguousarray(W2.T.astype(np.float32))  # (K, M*M)
    w2t = np.ascontiguousarray(
        W2T.reshape(KC, M, QG, qg).transpose(2, 1, 0, 3).astype(bf)
    )
    w_mapT = np.ascontiguousarray(
        W_map.T.astype(np.float32).reshape(VC, M, M).transpose(1, 0, 2).astype(bf)
    )
    # packed small weights: [w_map | w1T | b2r(f32) | b1c(f32)] per partition
    w_map_bf = np.ascontiguousarray(W_map.astype(bf))
    w1T_bf = np.ascontiguousarray(W1.T.astype(bf))
    b2r_f = np.ascontiguousarray(b2.astype(np.float32).reshape(M, M).T)
    b1c_f = np.ascontiguousarray(b1.astype(np.float32).reshape(K // M, M).T)
    wpk = np.concatenate(
        [w_map_bf, w1T_bf, b2r_f.view(bf), b1c_f.view(bf)], axis=1
    )
    shared = {
        "w_mapT": w_mapT,
        "wpk": np.ascontiguousarray(wpk),
        "w2t": w2t,
    }
    in_maps = []
    for c in range(n_cores):
        shard = xf[c * n_rows : (c + 1) * n_rows]  # (n_rows, V)
        xT = shard.T  # (V, n_rows)
        # xtb[p, nb, vc, n] = xT[vc*M + p, nb*NB + n]
        xtb = np.ascontiguousarray(
            xT.reshape(VC, M, nblk, NB).transpose(1, 2, 0, 3).astype(bf)
        )
        m = dict(shared)
        m["xtb"] = xtb
        in_maps.append(m)
    return in_maps


def assemble_out(results, n_rows):
    nblk = n_rows // NB
    VC = V // M
    outs = []
    for r in results:
        o = np.asarray(r["out"], dtype=np.float32)  # (nblk, VC, M, NB)
        # rows: nb*NB + n ; cols: vc*M + p
        outs.append(o.transpose(0, 3, 1, 2).reshape(n_rows, V))
    return np.concatenate(outs, axis=0)


_GRAPH_CACHE = {}


def _get_graph(n_rows):
    if n_rows not in _GRAPH_CACHE:
        _GRAPH_CACHE[n_rows] = build_graph(n_rows)
    return _GRAPH_CACHE[n_rows]


def kernel(x, W_map, W1, b1, W2, b2):
    from concourse.bass_utils import run_bass_kernel_spmd

    pre_shape = x.shape[:-1]
    xf = np.asarray(x, dtype=np.float32).reshape(-1, V)
    n_rows = xf.shape[0] // N_CORES
    nc = _get_graph(n_rows)
    in_maps = make_in_maps(xf, W_map, W1, b1, W2, b2)
    res = run_bass_kernel_spmd(nc, in_maps, core_ids=list(range(N_CORES)))
    return assemble_out(res.results, n_rows).reshape(*pre_shape, V)
